# revision 2
# baseline (speedup 1.0000x reference)
"""AmplitudeQuantumNet Trainium2 kernel v2 (8-core data parallel).

Pipeline per core (128 samples), chunked by 32 samples:
  conv1(1->32,3x3)+BN+bias -> relu+maxpool2        [K=19 im2col matmul, 4 jy blocks;
                                                    pool-x on DVE, pool-y writes into
                                                    interleaved-y in2 layout directly]
  conv2(32->64,3x3)+BN -> maxpool2 -> +bias,relu   [K=128 (4 x-shift blocks x 32ch),
                                                    M=128 (2 x-parity x 64ch): pool-x
                                                    happens in the PE array; x-shift
                                                    blocks built by 3 flat-shift DMAs]
  fc(3136->256)+tanh                                [K=128 (x-parity-split p2full), 56 matmuls]
  quantum statevector sim                           [host-built 256x256 unitary, 4 real matmuls]
  probs -> Z expvals (+norm via unitarity) -> MLP   [tiny matmuls]
"""

import sys

sys.path.insert(0, "/opt/trn_rl_repo")

import numpy as np
import ml_dtypes

BF16 = ml_dtypes.bfloat16

N_QUBITS = 8
Q_DEPTH = 10
DIM = 256
BN_EPS = 1e-5
B = 1024
NCORES = 8
B_CORE = B // NCORES  # 128
CH = 32               # samples per chunk
NCHUNK = B_CORE // CH  # 4
JY = [0, 2, 1, 3]     # conv1 partition-block -> jy shift (pool pairs at +-64)

_CACHE = {}


# ---------------------------------------------------------------- host precompute
def _quantum_unitary(q_params):
    """256x256 complex matrix of the full circuit (H layer + 10x[RX layer + diag])."""
    bits = ((np.arange(DIM)[:, None] >> (N_QUBITS - 1 - np.arange(N_QUBITS))) & 1)
    ph = np.where(np.arange(N_QUBITS) % 2 == 0, 1j, np.exp(1j * np.pi / 4))
    diag = np.prod(np.power(ph[None, :], bits), axis=1)
    cz = np.ones(DIM)
    for i, j in [(0, 1), (2, 3), (4, 5), (6, 7), (1, 2), (3, 4), (5, 6)]:
        cz = cz * ((-1.0) ** (bits[:, i] * bits[:, j]))
    diagc = (diag * cz).astype(np.complex128)

    def app(M, U, w):
        M = M.reshape((2,) * N_QUBITS + (DIM,))
        M = np.moveaxis(M, w, 0)
        M = np.tensordot(U, M, axes=(1, 0))
        M = np.moveaxis(M, 0, w)
        return M.reshape(DIM, DIM)

    M = np.eye(DIM, dtype=np.complex128)
    H = np.array([[1.0, 1.0], [1.0, -1.0]]) / np.sqrt(2.0)
    for w in range(N_QUBITS):
        M = app(M, H, w)
    qw = np.asarray(q_params, np.float64).reshape(Q_DEPTH, N_QUBITS)
    X = np.array([[0.0, 1.0], [1.0, 0.0]])
    I2 = np.eye(2)
    for layer in range(Q_DEPTH):
        for w in range(N_QUBITS):
            t = qw[layer, w]
            U = np.cos(t / 2) * I2 - 1j * np.sin(t / 2) * X
            M = app(M, U, w)
        M = diagc[:, None] * M
    zsigns = (1 - 2 * bits).astype(np.float64)  # [256, 8]
    return M, zsigns


def _host_prep(inputs):
    f32 = np.float32
    x = np.asarray(inputs["x"], f32)  # [1024,1,28,28]

    inv1 = inputs["bn1_gamma"] / np.sqrt(inputs["bn1_var"] + BN_EPS)
    w1f = np.asarray(inputs["conv1_w"], f32) * inv1[:, None, None, None]
    b1f = (inputs["conv1_b"] - inputs["bn1_mean"]) * inv1 + inputs["bn1_beta"]
    inv2 = inputs["bn2_gamma"] / np.sqrt(inputs["bn2_var"] + BN_EPS)
    w2f = np.asarray(inputs["conv2_w"], f32) * inv2[:, None, None, None]
    b2f = (inputs["conv2_b"] - inputs["bn2_mean"]) * inv2 + inputs["bn2_beta"]

    # conv1 lhsT [19, 128]: rows (r6,dxc) + bias row; cols (jyblk, c)
    W1 = np.zeros((19, 128), f32)
    for blk in range(4):
        jy = JY[blk]
        for r6 in range(6):
            dy = r6 - jy
            if 0 <= dy <= 2:
                for dxc in range(3):
                    W1[r6 * 3 + dxc, blk * 32:(blk + 1) * 32] = w1f[:, 0, dy, dxc]
        W1[18, blk * 32:(blk + 1) * 32] = b1f
    W1 = np.ascontiguousarray(W1, f32).astype(BF16)

    # x im2col [1024, 19, 7, 32]: row p=(r6,dxc): xpad[s, 4q+r6, x+dxc] at cols 2:30;
    # cols 0,1,30,31 are zero (pool-x pad pairs), ALSO in the bias row so the
    # padded pooled columns come out exactly 0 after relu.
    xp = np.zeros((B, 30, 30), f32)
    xp[:, 1:29, 1:29] = x[:, 0]
    xim = np.zeros((B, 19, 7, 32), f32)
    for r6 in range(6):
        for dxc in range(3):
            xim[:, r6 * 3 + dxc, :, 2:30] = xp[:, r6:r6 + 25:4, dxc:dxc + 28]
    xim[:, 18, :, 2:30] = 1.0
    # per-core: 4 chunks packed into 128 partitions (chunk ci's 19 K-rows at
    # partitions 32ci..32ci+18) so each load spans >=64 partitions and the
    # DMA spreads across SDMA engines (small-partition transfers serialize
    # through one engine)
    xim_cores = []
    for i in range(NCORES):
        xi = xim[i * B_CORE:(i + 1) * B_CORE].transpose(1, 0, 2, 3)  # [19, 128, 7, 32]
        xc = np.zeros((128, CH, 7, 32), np.float32)
        for ci in range(NCHUNK):
            xc[32 * ci:32 * ci + 19] = xi[:, ci * CH:(ci + 1) * CH]
        xim_cores.append(np.ascontiguousarray(xc).astype(BF16))

    # conv2 lhsT [128, 3, 128]: rows k=(b4, ci32) (b = x-shift block: in col 2t+b),
    # cols m=(px2, co64) (out col 2t+px). W2v2[(b,ci), dy, (px,co)] = w2f[co,ci,dy,b-px]
    W2 = np.zeros((128, 3, 128), f32)
    for bblk in range(4):
        for px in range(2):
            dxp = bblk - px
            if 0 <= dxp <= 2:
                for dy in range(3):
                    # [co, ci] -> rows bblk*32+ci, cols px*64+co
                    W2[bblk * 32:(bblk + 1) * 32, dy, px * 64:(px + 1) * 64] = \
                        w2f[:, :, dy, dxp].T
    W2 = np.ascontiguousarray(W2).astype(BF16)

    # fc lhsT [128, 28, 2, 128]: K=(pxf2, c64); pixel groups pg=(Xp4, Y7): X=2*Xp+pxf
    fcwr = np.asarray(inputs["fc_w"], f32).reshape(2, 128, 64, 7, 7)  # [mt, m, c, Y, X]
    fcw2 = np.zeros((2, 64, 4, 7, 2, 128), f32)  # [pxf, c, Xp, Y, mt, m]
    for pxf in range(2):
        for Xp in range(4):
            Xc = 2 * Xp + pxf
            if Xc < 7:
                fcw2[pxf, :, Xp] = fcwr[:, :, :, :, Xc].transpose(2, 3, 0, 1)
    fcw2 = np.ascontiguousarray(
        fcw2.reshape(128, 28, 2, 128)).astype(BF16)
    fcb = np.asarray(inputs["fc_b"], f32).reshape(2, 128).T  # [128, 2]
    fcb = np.ascontiguousarray(fcb)

    M, zsigns = _quantum_unitary(np.asarray(inputs["q_params"], np.float64))
    # lhsT tiles [k128, kb2, mt2, m128]: value M[mt*128+m, kb*128+k]
    mrt = M.real.T.reshape(2, 128, 2, 128).transpose(1, 0, 2, 3)
    mit = M.imag.T.reshape(2, 128, 2, 128).transpose(1, 0, 2, 3)
    mrt = np.ascontiguousarray(mrt).astype(f32).astype(BF16)
    mit = np.ascontiguousarray(mit).astype(f32).astype(BF16)
    zext = np.ones((DIM, 9), np.float64)
    zext[:, :8] = zsigns
    zext = np.ascontiguousarray(zext.reshape(2, 128, 9).transpose(1, 0, 2)).astype(f32).astype(BF16)

    p1t = np.ascontiguousarray(np.asarray(inputs["p1_w"], f32).T).astype(BF16)  # [8,128]
    p2t = np.ascontiguousarray(np.asarray(inputs["p2_w"], f32).T).astype(BF16)  # [128,64]
    p3t = np.ascontiguousarray(np.asarray(inputs["p3_w"], f32).T).astype(BF16)  # [64,10]

    common = {
        "w1": W1, "w2": W2, "fcw": fcw2, "fcb": fcb,
        "mrt": mrt, "mit": mit, "zext": zext,
        "p1t": p1t, "p2t": p2t, "p3t": p3t,
        "cb2a": np.asarray(b2f, f32).reshape(64, 1),
        "cb2b": np.asarray(b2f, f32).reshape(64, 1),
        "pb1": np.asarray(inputs["p1_b"], f32).reshape(128, 1),
        "pb2": np.asarray(inputs["p2_b"], f32).reshape(64, 1),
        "pb3": np.asarray(inputs["p3_b"], f32).reshape(10, 1),
    }
    in_maps = []
    for i in range(NCORES):
        m = dict(common)
        m["xim"] = xim_cores[i]
        in_maps.append(m)
    return in_maps


# ---------------------------------------------------------------- bass program
def _build_bass():
    import concourse.bacc as bacc
    import concourse.mybir as mybir
    import concourse.tile as tile

    dt = mybir.dt
    AF = mybir.ActivationFunctionType
    ALU = mybir.AluOpType

    nc = bacc.Bacc("TRN2", target_bir_lowering=False, debug=False,
                   num_devices=NCORES)
    xim = nc.dram_tensor("xim", [128, CH, 7, 32], dt.bfloat16, kind="ExternalInput")
    w1 = nc.dram_tensor("w1", [19, 128], dt.bfloat16, kind="ExternalInput")
    w2 = nc.dram_tensor("w2", [128, 3, 128], dt.bfloat16, kind="ExternalInput")
    fcw = nc.dram_tensor("fcw", [128, 28, 2, 128], dt.bfloat16, kind="ExternalInput")
    fcb = nc.dram_tensor("fcb", [128, 2], dt.float32, kind="ExternalInput")
    mrt = nc.dram_tensor("mrt", [128, 2, 2, 128], dt.bfloat16, kind="ExternalInput")
    mit = nc.dram_tensor("mit", [128, 2, 2, 128], dt.bfloat16, kind="ExternalInput")
    zext = nc.dram_tensor("zext", [128, 2, 9], dt.bfloat16, kind="ExternalInput")
    p1t = nc.dram_tensor("p1t", [8, 128], dt.bfloat16, kind="ExternalInput")
    p2t = nc.dram_tensor("p2t", [128, 64], dt.bfloat16, kind="ExternalInput")
    p3t = nc.dram_tensor("p3t", [64, 10], dt.bfloat16, kind="ExternalInput")
    cb2a = nc.dram_tensor("cb2a", [64, 1], dt.float32, kind="ExternalInput")
    cb2b = nc.dram_tensor("cb2b", [64, 1], dt.float32, kind="ExternalInput")
    pb1 = nc.dram_tensor("pb1", [128, 1], dt.float32, kind="ExternalInput")
    pb2 = nc.dram_tensor("pb2", [64, 1], dt.float32, kind="ExternalInput")
    pb3 = nc.dram_tensor("pb3", [10, 1], dt.float32, kind="ExternalInput")
    out = nc.dram_tensor("out", [10, B_CORE], dt.float32, kind="ExternalOutput")

    with tile.TileContext(nc) as tc:
        with tc.tile_pool(name="singles", bufs=1) as singles:
            # needed first: conv weights + biases. w1 is replicated at
            # partition offsets 0/32/64/96 so each chunk's conv1 matmul can
            # read its xim slab from a different partition range (each SDMA
            # engine serves a fixed 8-partition slice of SBUF; a 19-partition
            # tensor otherwise funnels through one engine).
            w1_sb = singles.tile([128, 128], dt.bfloat16, tag="w1")
            for ci in range(NCHUNK):
                nc.sync.dma_start(out=w1_sb[32 * ci:32 * ci + 19], in_=w1[:, :])
            w2_sb = singles.tile([128, 3, 128], dt.bfloat16, tag="w2")
            nc.sync.dma_start(out=w2_sb, in_=w2[:, :, :])
            cb2a_sb = singles.tile([64, 1], dt.float32, tag="cb2a")
            nc.sync.dma_start(out=cb2a_sb, in_=cb2a[:, :])
            cb2b_sb = singles.tile([64, 1], dt.float32, tag="cb2b")
            nc.sync.dma_start(out=cb2b_sb, in_=cb2b[:, :])
            # chunk input slabs at partition offsets 0/32/64/96 of ONE tile so
            # the loads use different SDMA engines
            xim_all = singles.tile([128, CH, 7, 32], dt.bfloat16, tag="xim_all")
            xim_sb = [xim_all[32 * ci:32 * ci + 19] for ci in range(NCHUNK)]
            nc.sync.dma_start(out=xim_all[0:64], in_=xim[0:64, :, :, :])
            nc.sync.dma_start(out=xim_all[64:128], in_=xim[64:128, :, :, :])
            # tail-weight tiles; their DMAs are issued mid-loop so the initial
            # queue drain doesn't delay conv1's inputs
            fcw_sb = singles.tile([128, 28, 2, 128], dt.bfloat16, tag="fcw")
            fcb_sb = singles.tile([128, 2], dt.float32, tag="fcb")
            mrt_sb = singles.tile([128, 2, 2, 128], dt.bfloat16, tag="mrt")
            mit_sb = singles.tile([128, 2, 2, 128], dt.bfloat16, tag="mit")
            zext_sb = singles.tile([128, 2, 9], dt.bfloat16, tag="zext")
            p1t_sb = singles.tile([8, 128], dt.bfloat16, tag="p1t")
            p2t_sb = singles.tile([128, 64], dt.bfloat16, tag="p2t")
            p3t_sb = singles.tile([64, 10], dt.bfloat16, tag="p3t")
            pb1_sb = singles.tile([128, 1], dt.float32, tag="pb1")
            pb2_sb = singles.tile([64, 1], dt.float32, tag="pb2")
            pb3_sb = singles.tile([10, 1], dt.float32, tag="pb3")
            ones18 = singles.tile([1, 8], dt.bfloat16, tag="ones18")
            nc.vector.memset(ones18, 1.0)

            # fc input, x-parity split: partitions (pxf2, c64); free [s, Xp4, Y7]
            p2full = singles.tile([128, B_CORE, 4, 7], dt.bfloat16, tag="p2full")
            # (pxf=1, Xp=3) block is never written; zero it so fc matmul
            # reads 0 (weights there are 0 too, but NaN*0 = NaN)
            nc.vector.memset(p2full[64:128, :, 3:4, :], 0.0)

            # ---------------- conv pipeline ----------------
            with tc.tile_pool(name="c1ps", bufs=2, space="PSUM") as c1pool, \
                 tc.tile_pool(name="c2ps", bufs=2, space="PSUM") as c2pool, \
                 tc.tile_pool(name="o1p", bufs=4) as o1pool, \
                 tc.tile_pool(name="p1cp", bufs=3) as p1cpool, \
                 tc.tile_pool(name="rcp", bufs=3) as rcpool, \
                 tc.tile_pool(name="in2p", bufs=3) as in2pool, \
                 tc.tile_pool(name="tod", bufs=4) as todpool, \
                 tc.tile_pool(name="t3p", bufs=4) as t3pool:
                for ci in range(NCHUNK):
                    # -------- conv1 + relu + pool-x --------
                    p1c = p1cpool.tile([128, CH, 7, 16], dt.bfloat16, tag="p1c")
                    for tt in range(8):  # subtiles of 4 samples
                        c1p = c1pool.tile([128, 2, 512], dt.float32, tag="c1p", name="c1p")
                        pos = 32 * ci
                        for sh in range(2):
                            nc.tensor.matmul(
                                c1p[:, sh, 0:448].rearrange(
                                    "p (s q x) -> p s q x", s=2, q=7, x=32),
                                w1_sb[pos:pos + 19],
                                xim_sb[ci][:, tt * 4 + sh * 2: tt * 4 + (sh + 1) * 2],
                                start=True, stop=True,
                                tile_position=(pos, 0))
                        c1v = c1p[:, :, 0:448].rearrange(
                            "p h (s q xp two) -> p h s q xp two", s=2, q=7, xp=16, two=2)
                        o1v = o1pool.tile([128, 2, 2, 7, 16], dt.bfloat16, tag="o1v")
                        nc.scalar.activation(o1v, c1v[:, :, :, :, :, 1], AF.Copy)
                        nc.vector.scalar_tensor_tensor(
                            p1c[:, tt * 4:(tt + 1) * 4].rearrange(
                                "p (h s) q xp -> p h s q xp", h=2),
                            c1v[:, :, :, :, :, 0], 0.0, o1v, ALU.max, ALU.max)

                    # -------- pool-y -> in2 block0 (interleaved-y layout) --------
                    # in2 [128, ypad16, s32, x16]; block0 = partitions 0:32
                    in2 = in2pool.tile([128, 16, CH, 16], dt.bfloat16, tag="in2")
                    nc.vector.memset(in2[0:32, 0:1, :, :], 0.0)
                    nc.vector.memset(in2[0:32, 15:16, :, :], 0.0)
                    r_c = rcpool.tile([64, CH, 7, 16], dt.bfloat16, tag="r_c")
                    nc.sync.dma_start(
                        out=r_c.rearrange("p s q x -> p (s q x)").rearrange(
                            "p (a b) -> p a b", b=896),
                        in_=p1c[64:128].rearrange("p s q x -> p (s q x)").rearrange(
                            "p (a b) -> p a b", b=896))
                    # x pads (cols 0,15) flow in as zeros from p1c's padded cols.
                    # pooled row r even -> ypad = r+1 odd (1,3,..,13)
                    oddrows = in2[0:32, 1:15].rearrange(
                        "p (q two) s x -> p q two s x", two=2)[:, :, 0, :, :]
                    nc.vector.tensor_tensor(
                        oddrows, p1c[0:32].rearrange("p s q x -> p q s x"),
                        r_c[0:32].rearrange("p s q x -> p q s x"), ALU.max)
                    # pooled row r odd -> ypad = r+1 even (2,4,..,14)
                    evenrows = in2[0:32, 2:16].rearrange(
                        "p (q two) s x -> p q two s x", two=2)[:, :, 0, :, :]
                    nc.vector.tensor_tensor(
                        evenrows, p1c[32:64].rearrange("p s q x -> p q s x"),
                        r_c[32:64].rearrange("p s q x -> p q s x"), ALU.max)

                    # -------- x-shift blocks via flat-shift DMAs --------
                    # block b at partitions 32b:32b+32 holds in_pad[x + b]
                    f0 = in2[0:32].rearrange("p y s x -> p (y s x)")
                    for bblk in range(1, 4):
                        dstf = in2[32 * bblk:32 * (bblk + 1)].rearrange(
                            "p y s x -> p (y s x)")
                        for j in range(4):
                            d0 = j * 2048 if j < 3 else 6144 - bblk
                            s0 = d0 + bblk
                            nc.sync.dma_start(
                                out=dstf[:, d0:d0 + 2048].rearrange(
                                    "p (a b) -> p a b", b=512),
                                in_=f0[:, s0:s0 + 2048].rearrange(
                                    "p (a b) -> p a b", b=512))

                    # -------- conv2 (pool-x in PE) + pool-y + bias + relu --------
                    in2f = in2.rearrange("p y s x -> p (y s x)")
                    for tl in range(4):  # tiles of 2 sample-quads (8 samples)
                        c2p = c2pool.tile([128, 2, 512], dt.float32, tag="c2p", name="c2p")
                        for g2 in range(2):
                            g = tl * 2 + g2  # sample quad in chunk
                            for dy in range(3):
                                rhs = in2f[:, dy * 512: dy * 512 + 7168].rearrange(
                                    "p (j2 jp s xh t) -> p j2 jp s xh t",
                                    j2=7, jp=2, s=CH, xh=8, t=2)[
                                        :, :, :, g * 4:g * 4 + 4, 0:7, 0]
                                nc.tensor.matmul(
                                    c2p[:, g2, 0:392].rearrange(
                                        "p (s j2 jp t) -> p s j2 jp t",
                                        s=4, j2=7, jp=2, t=7),
                                    w2_sb[:, dy, :],
                                    rhs.rearrange("p j2 jp s t -> p s j2 jp t"),
                                    start=(dy == 0), stop=(dy == 2))
                        # odd x-parity half -> SBUF (one-PSUM-operand rule)
                        t_odd = todpool.tile([64, 2, 392], dt.bfloat16, tag="t_odd")
                        nc.scalar.activation(t_odd, c2p[64:128, :, 0:392], AF.Copy)
                        t2 = t3pool.tile([64, 2, 392], dt.bfloat16, tag="t2")
                        nc.vector.tensor_tensor(t2, c2p[0:64, :, 0:392], t_odd, ALU.max)
                        t2v = t2.rearrange("p g (s Y yp t) -> p g s Y yp t",
                                           s=4, Y=7, yp=2, t=7)
                        t3 = t3pool.tile([64, 2, 4, 7, 7], dt.bfloat16, tag="t3")  # [p,g,s,t,Y]
                        nc.vector.tensor_tensor(
                            t3,
                            t2v[:, :, :, :, 0, :].rearrange("p g s Y t -> p g s t Y"),
                            t2v[:, :, :, :, 1, :].rearrange("p g s Y t -> p g s t Y"),
                            ALU.max)
                        t3v = t3.rearrange("p g s t Y -> p (g s) t Y")
                        smp0 = ci * CH + tl * 8
                        nc.scalar.activation(
                            p2full[0:64, smp0:smp0 + 8, :, :],
                            t3v[:, :, 0::2, :], AF.Relu, bias=cb2a_sb[:, 0:1])
                        nc.scalar.activation(
                            p2full[64:128, smp0:smp0 + 8, 0:3, :],
                            t3v[:, :, 1::2, :], AF.Relu, bias=cb2b_sb[:, 0:1])

                    # stagger the tail-weight loads into the gaps after the
                    # early chunks' conv work is enqueued
                    if ci == 0:
                        nc.sync.dma_start(out=fcw_sb, in_=fcw[:, :, :, :])
                    elif ci == 1:
                        nc.sync.dma_start(out=mrt_sb, in_=mrt[:, :, :, :])
                        nc.sync.dma_start(out=mit_sb, in_=mit[:, :, :, :])
                    elif ci == 2:
                        nc.sync.dma_start(out=fcb_sb, in_=fcb[:, :])
                        nc.sync.dma_start(out=zext_sb, in_=zext[:, :, :])
                        nc.sync.dma_start(out=p1t_sb, in_=p1t[:, :])
                        nc.sync.dma_start(out=p2t_sb, in_=p2t[:, :])
                        nc.sync.dma_start(out=p3t_sb, in_=p3t[:, :])
                        nc.sync.dma_start(out=pb1_sb, in_=pb1[:, :])
                        nc.sync.dma_start(out=pb2_sb, in_=pb2[:, :])
                        nc.sync.dma_start(out=pb3_sb, in_=pb3[:, :])

            # ---------------- dense tail ----------------
            with tc.tile_pool(name="tail", bufs=1) as tail, \
                 tc.tile_pool(name="psumT", bufs=1, space="PSUM") as psumT:
                fp = psumT.tile([128, 2, 128], dt.float32, tag="fp")
                # transpose once so every fc matmul reads contiguous columns
                # (a stride-28 rhs costs ~3ns/col on the PE's fetch path)
                p2ft = tail.tile([128, 4, 7, 128], dt.bfloat16, tag="p2ft")
                nc.vector.tensor_copy(out=p2ft, in_=p2full.rearrange("p s xp Y -> p xp Y s"))
                prhs = p2ft.rearrange("p xp Y s -> p (xp Y) s")
                for mt in range(2):
                    for pg in range(28):
                        nc.tensor.matmul(
                            fp[:, mt], fcw_sb[:, pg, mt, :], prhs[:, pg, :],
                            start=(pg == 0), stop=(pg == 27))
                feats = tail.tile([128, 2, 128], dt.bfloat16, tag="feats")
                for mt in range(2):
                    nc.scalar.activation(feats[:, mt], fp[:, mt], AF.Tanh,
                                         bias=fcb_sb[:, mt:mt + 1])

                sq = psumT.tile([128, 4, 128], dt.float32, tag="sq")
                srp = sq[:, 0:2]
                sip = sq[:, 2:4]
                for mt in range(2):
                    for kb in range(2):
                        nc.tensor.matmul(srp[:, mt], mrt_sb[:, kb, mt, :], feats[:, kb],
                                         start=(kb == 0), stop=(kb == 1))
                    for kb in range(2):
                        nc.tensor.matmul(sip[:, mt], mit_sb[:, kb, mt, :], feats[:, kb],
                                         start=(kb == 0), stop=(kb == 1))

                t1 = tail.tile([128, 2, 128], dt.float32, tag="sq_r")
                nc.scalar.activation(t1, srp, AF.Square)
                t2s = tail.tile([128, 2, 128], dt.float32, tag="sq_i")
                nc.scalar.activation(t2s, sip, AF.Square)
                probs = tail.tile([128, 2, 128], dt.bfloat16, tag="probs")
                nc.vector.tensor_tensor(probs, t1, t2s, ALU.add)

                qt = psumT.tile([8, 2, 128], dt.float32, tag="qt")
                qp = qt[:, 0]
                tp = qt[0:1, 1]
                for kb in range(2):
                    nc.tensor.matmul(qp, zext_sb[:, kb, 0:8], probs[:, kb],
                                     start=(kb == 0), stop=(kb == 1))
                for kb in range(2):
                    nc.tensor.matmul(tp, zext_sb[:, kb, 8:9], probs[:, kb],
                                     start=(kb == 0), stop=(kb == 1))

                recip = tail.tile([1, 128], dt.float32, tag="recip")
                nc.vector.reciprocal(recip, tp)
                recip_bf = tail.tile([1, 128], dt.bfloat16, tag="recip_bf")
                nc.vector.tensor_copy(out=recip_bf, in_=recip)
                bc = psumT.tile([8, 128], dt.float32, tag="bc")
                nc.tensor.matmul(bc, ones18, recip_bf, start=True, stop=True)
                bc_sb = tail.tile([8, 128], dt.float32, tag="bc_sb")
                nc.scalar.activation(bc_sb, bc, AF.Copy)

                qn = tail.tile([8, 128], dt.bfloat16, tag="qn")
                nc.vector.tensor_tensor(qn, qp[0:8, :], bc_sb, ALU.mult)

                zp = psumT.tile([128, 3, 128], dt.float32, tag="zp")
                z1p = zp[:, 0]
                z2p = zp[0:64, 1]
                z3p = zp[0:10, 2]
                nc.tensor.matmul(z1p, p1t_sb, qn, start=True, stop=True)
                z1 = tail.tile([128, 128], dt.bfloat16, tag="z1")
                nc.scalar.activation(z1, z1p, AF.Relu, bias=pb1_sb[:, 0:1])

                nc.tensor.matmul(z2p, p2t_sb, z1, start=True, stop=True)
                z2 = tail.tile([64, 128], dt.bfloat16, tag="z2")
                nc.scalar.activation(z2, z2p, AF.Relu, bias=pb2_sb[:, 0:1])

                nc.tensor.matmul(z3p, p3t_sb, z2, start=True, stop=True)
                osb = tail.tile([10, 128], dt.float32, tag="osb")
                nc.vector.tensor_scalar_add(osb, z3p, pb3_sb[:, 0:1])
                nc.sync.dma_start(out=out[:, :], in_=osb)

    nc.finalize()
    return nc


def _get_nc():
    if "nc" not in _CACHE:
        _CACHE["nc"] = _build_bass()
    return _CACHE["nc"]


def kernel(**inputs) -> np.ndarray:
    from concourse.bass_utils import run_bass_kernel_spmd

    in_maps = _host_prep(inputs)
    nc = _get_nc()
    res = run_bass_kernel_spmd(nc, in_maps, core_ids=list(range(NCORES)),
                               trace=bool(_CACHE.get("trace")))
    _CACHE["last_result"] = res
    outs = [r["out"].T for r in res.results]  # each [128, 10]
    return np.ascontiguousarray(np.concatenate(outs, axis=0), dtype=np.float32)
